# revision 14
# baseline (speedup 1.0000x reference)
"""Bass/Trainium2 kernel for a ragged-sequence CrossAttentionBlock.

Math (per reference):
  T = 16*196 packed tokens, D=512, H=8 heads of HD=64.
  q = (xq + pos) @ Wq + bq ; k = (xk + pos) @ Wk + bk ; v = xk @ Wv + bv
  block-diagonal attention over segments of channels[i]*196 tokens
  out = softmax(q k^T / 8) v  -> concat heads -> @ Wo + bo

Sharding: one head per NeuronCore. Each core computes its head's
Q/K/V over all (padded) tokens, the per-segment attention, and its
head's slice of the output projection Wo[h*64:(h+1)*64, :].

Bias folding (exact):
  bk: adds q.bk to every logit of a query row -> softmax-invariant, drops.
  bq: adds bq.k_j per key -> equivalent to scaling V rows (and the ones
      column) by exp(bq.k_j/8); host computes the factors (it knows xkp and
      Wk) and a program variant applies them on device. Zero biases (the
      spec's fill) use the fast variant with no scaling.
  bv, bo: commute through softmax (rows sum to 1): host adds bv@Wo + bo
      to the final output.

Device design notes:
  - x inputs are host-side pre-added (xqp = xq+pos, xkp = xk+pos) and
    pre-transposed into the exact SBUF layout (contiguous partition
    lines). xkp ships as fp8e4m3 (the K-side path tolerates it; measured
    output error ~1.4e-2 vs the 2e-2 gate), xqp/xk as bf16. The PE runs
    mixed bf16-weight x fp8-activation matmuls natively.
  - the whole kernel runs under ONE flat PSUM budget (no phase pools):
    score pools of 3+2 banks alternate for batched exp, one bank each for
    QK projections, V^T/outproj (shared), and AV accumulation. This lets
    attention for early segments overlap projections/DMA of later ones.
  - emission is segment-major (big segments first): each segment's Q/K
    blocks stream in, then its xk blocks; exp for segment s runs while
    segment s+1 still loads.
  - Q/K projections run as col-packed concurrent matmul pairs; V^T
    projections col-pack two token blocks; scores row-pack two k-tiles
    (K^T tiles alternate partition halves, Q^T is duplicated); the output
    projection row-packs two query blocks against a duplicated Wo.
  - V^T -> V uses the DMA XBAR transpose, not the PE.
  - softmax denominators ride as a ones-column through the AV matmul;
    normalization happens on the host (outputs are unnormalized projected
    values plus per-token sums).
  - the output projection for a query-block pair is emitted as soon as
    its O^T halves land, so write-out overlaps attention.
  - a dummy-matmul warmup stream runs during the input DMA head to lift
    the PE HAM clock gate before real matmuls start.
"""

import sys
import types

import numpy as np
import ml_dtypes

_D = 512
_HD = 64
_H = 8
_S = 196
_NCORES = 8

_prog_cache = {}


def _ensure_ntff_hook():
    """Register the NTFF profile hook that the agent image's antenv lacks."""
    if "antenv.axon_hooks" in sys.modules:
        return
    try:
        from trn_agent_boot.trn_boot import _ntff_profile_via_ctypes

        hook = _ntff_profile_via_ctypes("/opt/axon/libaxon_pjrt.so")
        mod = types.ModuleType("antenv.axon_hooks")
        mod.get_axon_ntff_profile_hook = lambda: hook
        sys.modules["antenv.axon_hooks"] = mod
    except Exception:
        pass


def _segments(channels):
    """Return (seg_len, seg_pad, seg_off, TP) for the padded token axis."""
    seg_len = [int(c) * _S for c in np.asarray(channels).tolist() if int(c) > 0]
    seg_pad = [(l + 127) // 128 * 128 for l in seg_len]
    TP = sum(seg_pad)
    TP = (TP + 511) // 512 * 512
    seg_off = []
    o = 0
    for p in seg_pad:
        seg_off.append(o)
        o += p
    return seg_len, seg_pad, seg_off, TP


def _plan(seg_len, seg_pad, seg_off, TP):
    """Static schedule metadata shared by program build and host prep."""
    nseg = len(seg_len)
    NTB = TP // 512
    NKT = TP // 128
    nkt_s = [p // 128 for p in seg_pad]
    start_tile = [o // 128 for o in seg_off]

    g2seg = [None] * NKT
    for s in range(nseg):
        for lk in range(nkt_s[s]):
            g2seg[start_tile[s] + lk] = (s, lk)

    slot_base = []
    b = 0
    for s in range(nseg):
        slot_base.append(b)
        b += (nkt_s[s] + 1) // 2
    NSLOT = b

    w = [seg_pad[s] * seg_len[s] for s in range(nseg)]
    seg_order = sorted(range(nseg), key=lambda s: -w[s])

    def blocks_of(s):
        lo = seg_off[s] // 512
        hi = (seg_off[s] + seg_pad[s] + 511) // 512
        return list(range(lo, hi))

    qblocks = []  # (seg, qb0, qbw)
    for s in seg_order:
        for qb0 in range(0, seg_len[s], 512):
            qblocks.append((s, qb0, min(512, seg_len[s] - qb0)))
    NQB = len(qblocks)
    NPAIR = (NQB + 1) // 2

    return dict(
        nseg=nseg, NTB=NTB, NKT=NKT, nkt_s=nkt_s, start_tile=start_tile,
        g2seg=g2seg, slot_base=slot_base, NSLOT=NSLOT, seg_order=seg_order,
        blocks_of=blocks_of, qblocks=qblocks, NQB=NQB, NPAIR=NPAIR,
    )


def _build_program(seg_key, bq_nonzero):
    import concourse.bacc as bacc
    import concourse.tile as tile
    from concourse import mybir

    seg_len, seg_pad, seg_off, TP = seg_key
    seg_len, seg_pad, seg_off = list(seg_len), list(seg_pad), list(seg_off)
    P = _plan(seg_len, seg_pad, seg_off, TP)
    nseg, NTB, NKT = P["nseg"], P["NTB"], P["NKT"]
    nkt_s, start_tile, g2seg = P["nkt_s"], P["start_tile"], P["g2seg"]
    slot_base, NSLOT = P["slot_base"], P["NSLOT"]
    seg_order, blocks_of = P["seg_order"], P["blocks_of"]
    qblocks, NPAIR = P["qblocks"], P["NPAIR"]

    f32 = mybir.dt.float32
    bf16 = mybir.dt.bfloat16
    f8e4 = mybir.dt.float8e4

    nc = bacc.Bacc("TRN2", target_bir_lowering=False, debug=False,
                   num_devices=_NCORES)

    xqp_d = nc.dram_tensor("xqp", [NTB, 128, 4, 512], bf16, kind="ExternalInput")
    xkp_d = nc.dram_tensor("xkp", [NTB, 128, 4, 512], f8e4, kind="ExternalInput")
    xk_d = nc.dram_tensor("xk", [NTB, 128, 4, 512], bf16, kind="ExternalInput")
    wq_d = nc.dram_tensor("wq", [128, 4, _HD], bf16, kind="ExternalInput")
    wk_d = nc.dram_tensor("wk", [128, 4, _HD], bf16, kind="ExternalInput")
    wv_d = nc.dram_tensor("wv", [128, 4, _HD], bf16, kind="ExternalInput")
    wo2_d = nc.dram_tensor("wo2", [128, 4, 128], bf16, kind="ExternalInput")
    if bq_nonzero:
        vscale_d = nc.dram_tensor("vscale", [128, NKT], f32, kind="ExternalInput")
    outT_d = nc.dram_tensor("outT", [_D, TP], bf16, kind="ExternalOutput")
    sums_d = nc.dram_tensor("sums", [1, TP], f32, kind="ExternalOutput")

    with tile.TileContext(nc) as tc:
        with (
            tc.tile_pool(name="consts", bufs=1) as consts,
            tc.tile_pool(name="persist", bufs=1) as persist,
            tc.tile_pool(name="xin", bufs=4) as xin,
            tc.tile_pool(name="sbw", bufs=2) as sbw,
            tc.tile_pool(name="expp", bufs=3) as expp,
            tc.tile_pool(name="fins", bufs=4) as fins,
            tc.tile_pool(name="pqk", bufs=1, space="PSUM") as pqk,
            tc.tile_pool(name="pvf", bufs=1, space="PSUM") as pvf,
            tc.tile_pool(name="scA", bufs=1, space="PSUM") as scA,
            tc.tile_pool(name="scB", bufs=1, space="PSUM") as scB,
            tc.tile_pool(name="ops", bufs=1, space="PSUM") as opsp,
        ):
            wq_sb = consts.tile([128, 4, _HD], bf16)
            wk_sb = consts.tile([128, 4, _HD], bf16)
            wv_sb = consts.tile([128, 4, _HD], bf16)
            wo2_sb = consts.tile([128, 4, 128], bf16)
            nc.sync.dma_start(out=wq_sb, in_=wq_d[:, :, :])
            nc.sync.dma_start(out=wk_sb, in_=wk_d[:, :, :])
            nc.sync.dma_start(out=wv_sb, in_=wv_d[:, :, :])
            nc.sync.dma_start(out=wo2_sb, in_=wo2_d[:, :, :])
            if bq_nonzero:
                vscale_sb = consts.tile([128, NKT], f32)
                nc.sync.dma_start(out=vscale_sb, in_=vscale_d[:, :])
            wdummy = consts.tile([128, 128], bf16)
            nc.vector.memset(wdummy, 0.125)

            # persistent per-head tensors
            q2_sb = persist.tile([128, TP], bf16)          # Q^T duplicated halves
            k2_sb = persist.tile([128, NSLOT, 128], bf16)  # K^T tiles, paired halves
            v_all = persist.tile([128, NKT, _HD + 1], bf16)  # V rows + ones col
            o2_sb = persist.tile([128, NPAIR, 512], bf16)  # unnormalized O^T pairs
            sums_sb = persist.tile([1, TP], f32)

            # ones column (no data deps: disjoint from the V writes)
            for s in range(nseg):
                g0 = start_tile[s]
                nfull = seg_len[s] // 128
                rem = seg_len[s] - nfull * 128
                if nfull:
                    nc.vector.memset(v_all[:, g0:g0 + nfull, _HD:_HD + 1], 1.0)
                for lk in range(nfull, nkt_s[s]):
                    nc.vector.memset(v_all[:, g0 + lk, _HD:_HD + 1], 0.0)
                if rem:
                    nc.vector.memset(v_all[0:rem, g0 + nfull, _HD:_HD + 1], 1.0)

            # HAM warmup through the DMA head (rotates the V/fin psum slot)
            warm_ps = pvf.tile([128, 512], f32, tag="vtf", name="warm")
            for _ in range(30):
                nc.tensor.matmul(warm_ps[:, 0:128], lhsT=wdummy, rhs=wdummy,
                                 start=True, stop=True)

            # ---------------- emission helpers ----------------
            def emit_qk_block(tb):
                xq_t = xin.tile([128, 4, 512], bf16, tag="xq", name="xq")
                xkp_t = xin.tile([128, 4, 512], f8e4, tag="xkp", name="xkp")
                nc.sync.dma_start(out=xq_t, in_=xqp_d[tb])
                nc.sync.dma_start(out=xkp_t, in_=xkp_d[tb])
                ts = slice(tb * 512, (tb + 1) * 512)
                qk_ps = pqk.tile([128, 512], f32, tag="qkps", name="qkps")
                for c in range(4):
                    nc.tensor.matmul(
                        qk_ps[0:64, :], lhsT=wq_sb[:, c], rhs=xq_t[:, c],
                        start=(c == 0), stop=(c == 3),
                    )
                    nc.tensor.matmul(
                        qk_ps[64:128, :], lhsT=wk_sb[:, c], rhs=xkp_t[:, c],
                        start=(c == 0), stop=(c == 3),
                    )
                nc.vector.tensor_copy(out=q2_sb[0:64, ts], in_=qk_ps[0:64, :])
                nc.gpsimd.tensor_copy(out=q2_sb[64:128, ts], in_=q2_sb[0:64, ts])
                tiles = []
                for i in range(4):
                    g = tb * 4 + i
                    if g2seg[g] is None:
                        continue
                    s, lk = g2seg[g]
                    tiles.append((i, slot_base[s] + lk // 2, lk % 2))
                done = set()
                for idx, (i, slot, half) in enumerate(tiles):
                    if i in done:
                        continue
                    mate = next(
                        (j for (j, sl2, h2) in tiles[idx + 1:]
                         if j == i + 2 and h2 == half and sl2 == slot + 1),
                        None,
                    )
                    if mate is not None:
                        done.add(mate)
                        nc.vector.tensor_copy(
                            out=k2_sb[64 * half:64 * half + 64, slot:slot + 2, :],
                            in_=qk_ps[64:128, :].rearrange(
                                "p (b t) -> p b t", b=4, t=128)[:, i:i + 3:2, :],
                        )
                    else:
                        nc.vector.tensor_copy(
                            out=k2_sb[64 * half:64 * half + 64, slot, :],
                            in_=qk_ps[64:128, i * 128:(i + 1) * 128],
                        )

            def emit_vpair(ba, ta, bb, tb_):
                vt_ps = pvf.tile([128, 512], f32, tag="vtf", name="vtps")
                for c in range(4):
                    nc.tensor.matmul(
                        vt_ps[0:64, :], lhsT=wv_sb[:, c], rhs=ta[:, c],
                        start=(c == 0), stop=(c == 3),
                    )
                    nc.tensor.matmul(
                        vt_ps[64:128, :], lhsT=wv_sb[:, c], rhs=tb_[:, c],
                        start=(c == 0), stop=(c == 3),
                    )
                vt_sb = sbw.tile([128, 512], bf16, tag="vtsb", name="vtsb")
                nc.vector.tensor_copy(out=vt_sb, in_=vt_ps)
                stag = sbw.tile([128, 4, 128], bf16, tag="stag", name="stag")
                nc.sync.dma_start_transpose(stag, vt_sb)
                nc.gpsimd.tensor_copy(
                    out=v_all[:, 4 * ba:4 * ba + 4, 0:_HD], in_=stag[:, :, 0:64])
                nc.gpsimd.tensor_copy(
                    out=v_all[:, 4 * bb:4 * bb + 4, 0:_HD], in_=stag[:, :, 64:128])

            def emit_vlone(ba, ta):
                vt_ps = pvf.tile([128, 512], f32, tag="vtf", name="vtps")
                for c in range(4):
                    nc.tensor.matmul(
                        vt_ps[0:64, :], lhsT=wv_sb[:, c], rhs=ta[:, c],
                        start=(c == 0), stop=(c == 3),
                    )
                vt_sb1 = sbw.tile([64, 512], bf16, tag="vtsb1", name="vtsb1")
                nc.vector.tensor_copy(out=vt_sb1, in_=vt_ps[0:64, :])
                stag1 = sbw.tile([128, 4, 64], bf16, tag="stag1", name="stag1")
                nc.sync.dma_start_transpose(stag1, vt_sb1)
                nc.gpsimd.tensor_copy(
                    out=v_all[:, 4 * ba:4 * ba + 4, 0:_HD], in_=stag1[:, :, :])

            vpend = []

            def emit_xk_block(tb):
                xk_t = xin.tile([128, 4, 512], bf16, tag="xk", name="xk")
                nc.sync.dma_start(out=xk_t, in_=xk_d[tb])
                vpend.append((tb, xk_t))
                if len(vpend) == 2:
                    emit_vpair(vpend[0][0], vpend[0][1], vpend[1][0], vpend[1][1])
                    vpend.clear()

            def flush_vpend():
                # V must be emitted before attention that reads it: the shared
                # V/outproj psum slot chain would deadlock otherwise.
                if vpend:
                    emit_vlone(vpend[0][0], vpend[0][1])
                    vpend.clear()

            qb_meta = []  # (seg, qb0, qbw, goff, pslot, half)
            for i, (s, qb0, qbw) in enumerate(qblocks):
                qb_meta.append((s, qb0, qbw, seg_off[s] + qb0, i // 2, i % 2))

            def emit_outproj(pslot, members):
                f4 = {}
                for (s, qb0, qbw, goff, _, half) in members:
                    f4[half] = fins.tile([128, 4, 512], bf16, tag="fsb",
                                         name="fsb")
                for c in range(4):
                    for (s, qb0, qbw, goff, _, half) in members:
                        f_ps = pvf.tile([128, 512], f32, tag="vtf", name="fin")
                        nc.tensor.matmul(
                            f_ps[:, 0:qbw],
                            lhsT=wo2_sb[64 * half:64 * half + 64, c, :],
                            rhs=o2_sb[64 * half:64 * half + 64, pslot, 0:qbw],
                            start=True, stop=True,
                        )
                        nc.vector.tensor_copy(
                            out=f4[half][:, c, 0:qbw], in_=f_ps[:, 0:qbw])
                for (s, qb0, qbw, goff, _, half) in members:
                    nc.gpsimd.dma_start(
                        out=outT_d.rearrange("(c p) t -> p c t", p=128)[
                            :, :, goff:goff + qbw],
                        in_=f4[half][:, :, 0:qbw],
                    )

            exp_state = {"use_a": True}

            def emit_attention_qb(qi):
                s, qb0, qbw, goff, pslot, half = qb_meta[qi]
                nkt = nkt_s[s]
                g0 = start_tile[s]
                qcol = slice(goff, goff + qbw)
                ex = expp.tile([128, max(nkt_s), 512], bf16, tag="ex", name="ex")
                lk = 0
                while lk < nkt:
                    use_a = exp_state["use_a"]
                    cap = 3 if use_a else 2
                    G = min(cap, nkt - lk)
                    pool = scA if use_a else scB
                    sc = pool.tile(
                        [128, 3 if use_a else 2, 512], f32,
                        tag="scA" if use_a else "scB", name="sc",
                    )
                    exp_state["use_a"] = not use_a
                    for j in range(G):
                        l = lk + j
                        slot = slot_base[s] + l // 2
                        hh = l % 2
                        nc.tensor.matmul(
                            sc[:, j, 0:qbw],
                            lhsT=k2_sb[64 * hh:64 * hh + 64, slot, :],
                            rhs=q2_sb[64 * hh:64 * hh + 64, qcol],
                            start=True, stop=True,
                        )
                    nc.scalar.activation(
                        out=ex[:, lk:lk + G, 0:qbw],
                        in_=sc[:, 0:G, 0:qbw],
                        func=mybir.ActivationFunctionType.Exp,
                        scale=0.125,
                    )
                    lk += G
                # AV with ones column: O^T rows 0..63, sums in row 64
                o_ps = opsp.tile([_HD + 1, 512], f32, tag="ops", name="ops")
                for l in range(nkt):
                    nc.tensor.matmul(
                        o_ps[:, 0:qbw],
                        lhsT=v_all[:, g0 + l, :],
                        rhs=ex[:, l, 0:qbw],
                        start=(l == 0), stop=(l == nkt - 1),
                    )
                nc.vector.tensor_copy(
                    out=o2_sb[64 * half:64 * half + 64, pslot, 0:qbw],
                    in_=o_ps[0:_HD, 0:qbw],
                )
                nc.vector.tensor_copy(
                    out=sums_sb[0:1, qcol], in_=o_ps[_HD:_HD + 1, 0:qbw])
                if half == 1 or qi == len(qb_meta) - 1:
                    emit_outproj(pslot, [m for m in qb_meta if m[4] == pslot])

            # ---------------- emission stream ----------------
            qk_done = set()
            xk_done = set()
            qi = 0
            for s in seg_order:
                for tb in blocks_of(s):
                    if tb not in qk_done:
                        qk_done.add(tb)
                        emit_qk_block(tb)
                for tb in blocks_of(s):
                    if tb not in xk_done:
                        xk_done.add(tb)
                        emit_xk_block(tb)
                flush_vpend()
                if bq_nonzero:
                    for lk in range(nkt_s[s]):
                        g = start_tile[s] + lk
                        nc.vector.tensor_scalar_mul(
                            v_all[:, g, :], v_all[:, g, :], vscale_sb[:, g:g + 1])
                while qi < len(qb_meta) and qb_meta[qi][0] == s:
                    emit_attention_qb(qi)
                    qi += 1
            flush_vpend()
            while qi < len(qb_meta):
                emit_attention_qb(qi)
                qi += 1
            nc.gpsimd.dma_start(out=sums_d[:, :], in_=sums_sb[:, :])

    nc.compile()
    return nc


def _prep_blocked(x, seg_len, seg_pad, seg_off, TP, dtype):
    """[T, D] f32 -> [NTB, 128, 4, 512] with per-segment padding.

    Element (tb, p, c, t) = x[tb*512 + t (padded axis), c*128 + p].
    """
    xp = np.zeros((TP, _D), dtype=np.float32)
    o = 0
    for l, off in zip(seg_len, seg_off):
        xp[off:off + l] = x[o:o + l]
        o += l
    xt = xp.T.reshape(4, 128, TP // 512, 512)  # [c, p, tb, t]
    return np.ascontiguousarray(xt.transpose(2, 1, 0, 3)).astype(dtype)


def kernel(x_query, x_keyval, pos, channels, Wq, bq, Wk, bk, Wv, bv, Wo, bo,
           _trace=False, _trace_cores=None):
    _ensure_ntff_hook()
    import concourse.bass_utils as bu

    bu.upload_artifacts = lambda tmpdir: tmpdir  # no S3 egress from here

    x_query = np.asarray(x_query, dtype=np.float32)
    x_keyval = np.asarray(x_keyval, dtype=np.float32)
    pos = np.asarray(pos, dtype=np.float32)
    channels = np.asarray(channels)
    Wq, bq = np.asarray(Wq, np.float32), np.asarray(bq, np.float32)
    Wk, bk = np.asarray(Wk, np.float32), np.asarray(bk, np.float32)
    Wv, bv = np.asarray(Wv, np.float32), np.asarray(bv, np.float32)
    Wo, bo = np.asarray(Wo, np.float32), np.asarray(bo, np.float32)

    C, S, D = x_query.shape
    seg_len, seg_pad, seg_off, TP = _segments(channels)
    assert sum(seg_len) == C * S, "channels inconsistent with batch dim"
    P = _plan(seg_len, seg_pad, seg_off, TP)

    bq_nonzero = bool(np.any(bq))
    seg_key = (tuple(seg_len), tuple(seg_pad), tuple(seg_off), TP)
    cache_key = (seg_key, bq_nonzero)
    if cache_key not in _prog_cache:
        _prog_cache[cache_key] = _build_program(seg_key, bq_nonzero)
    nc = _prog_cache[cache_key]

    bf = ml_dtypes.bfloat16
    f8 = ml_dtypes.float8_e4m3fn
    xq_flat = x_query.reshape(-1, D)
    xk_flat = x_keyval.reshape(-1, D)
    p_flat = pos.reshape(-1, D)
    xqp_b = _prep_blocked(xq_flat + p_flat, seg_len, seg_pad, seg_off, TP, bf)
    xkp_b = _prep_blocked(xk_flat + p_flat, seg_len, seg_pad, seg_off, TP, f8)
    xk_b = _prep_blocked(xk_flat, seg_len, seg_pad, seg_off, TP, bf)

    def wchunk(w):  # [512, 64] -> [128, 4, 64]
        return np.ascontiguousarray(
            w.reshape(4, 128, _HD).transpose(1, 0, 2)).astype(bf)

    in_maps = []
    for h in range(_NCORES):
        sl = slice(h * _HD, (h + 1) * _HD)
        wo_h = Wo[sl, :].reshape(_HD, 4, 128)  # [64, c, j]
        wo2 = np.empty((128, 4, 128), dtype=np.float32)
        wo2[0:64] = wo_h
        wo2[64:128] = wo_h
        m = {
            "xqp": xqp_b,
            "xkp": xkp_b,
            "xk": xk_b,
            "wq": wchunk(Wq[:, sl]),
            "wk": wchunk(Wk[:, sl]),
            "wv": wchunk(Wv[:, sl]),
            "wo2": np.ascontiguousarray(wo2).astype(bf),
        }
        if bq_nonzero:
            # exact bq folding: scale V rows by exp((xkp @ Wk_h @ bq_h)/8)
            # (uses the same fp8-quantized xkp the device sees)
            xkp_q = (xk_flat + p_flat).astype(f8).astype(np.float32)
            bqk = xkp_q @ (
                Wk[:, sl].astype(bf).astype(np.float32) @ bq[sl])  # [T]
            f = np.zeros(TP, dtype=np.float64)
            o = 0
            for l, off in zip(seg_len, seg_off):
                f[off:off + l] = np.exp(bqk[o:o + l] / 8.0)
                o += l
            m["vscale"] = np.ascontiguousarray(
                f.reshape(P["NKT"], 128).T).astype(np.float32)
        in_maps.append(m)

    from concourse.bass_utils import run_bass_kernel_spmd

    kwargs = {}
    if _trace:
        kwargs["trace"] = True
        if _trace_cores is not None:
            kwargs["trace_cores"] = _trace_cores
    res = run_bass_kernel_spmd(nc, in_maps, list(range(_NCORES)), **kwargs)

    # host gather: per-head softmax normalization + sum + bias folds
    acc = np.zeros((_D, TP), dtype=np.float64)
    with np.errstate(divide="ignore", invalid="ignore"):
        for h in range(_NCORES):
            outT = np.asarray(res.results[h]["outT"], dtype=np.float64)
            sums = np.asarray(res.results[h]["sums"], dtype=np.float64)[0]
            sums = np.where(sums == 0.0, 1.0, sums)
            acc += outT / sums[None, :]

    const = bv @ Wo + bo  # bias fold (exact; zero in the spec's fills)

    out = np.empty((C * S, D), dtype=np.float32)
    o = 0
    for l, off in zip(seg_len, seg_off):
        out[o:o + l] = acc[:, off:off + l].T
        o += l
    out = (out + const[None, :]).astype(np.float32).reshape(C, S, D)

    if _trace:
        kernel._last_exec_time_ns = res.exec_time_ns
        kernel._last_trace = (
            res.instructions_and_trace[1] if res.instructions_and_trace else None
        )
    return out


# revision 19
# speedup vs baseline: 1.2797x; 1.2797x over previous
"""Bass/Trainium2 kernel for a ragged-sequence CrossAttentionBlock.

Math (per reference):
  T = 16*196 packed tokens, D=512, H=8 heads of HD=64.
  q = (xq + pos) @ Wq + bq ; k = (xk + pos) @ Wk + bk ; v = xk @ Wv + bv
  block-diagonal attention over segments of channels[i]*196 tokens
  out = softmax(q k^T / 8) v  -> concat heads -> @ Wo + bo

Sharding: one head per NeuronCore. Each core computes its head's
Q/K/V over all (padded) tokens, the per-segment attention, and its
head's slice of the output projection Wo[h*64:(h+1)*64, :].

Bias folding (exact):
  bk: adds q.bk to every logit of a query row -> softmax-invariant, drops.
  bq: adds bq.k_j per key -> equivalent to scaling V rows (and the ones
      column) by exp(bq.k_j/8); host computes the factors (it knows xkp and
      Wk) and a program variant applies them on device. Zero biases (the
      spec's fill) use the fast variant with no scaling.
  bv, bo: commute through softmax (rows sum to 1): host adds bv@Wo + bo
      to the final output.

Device design notes:
  - x inputs are host-side pre-added (xqp = xq+pos, xkp = xk+pos) and
    pre-transposed into the exact SBUF layout (contiguous partition
    lines). xkp ships as fp8e4m3 (the K-side path tolerates it; measured
    output error ~1.4e-2 vs the 2e-2 gate), xqp/xk as bf16. The PE runs
    mixed bf16-weight x fp8-activation matmuls natively.
  - the whole kernel runs under ONE flat PSUM budget (no phase pools):
    score pools of 3+2 banks alternate for batched exp, one bank each for
    QK projections, V^T/outproj (shared), and AV accumulation. This lets
    attention for early segments overlap projections/DMA of later ones.
  - emission is segment-major (big segments first): each segment's Q/K
    blocks stream in, then its xk blocks; exp for segment s runs while
    segment s+1 still loads.
  - Q/K projections run as col-packed concurrent matmul pairs; V^T
    projections col-pack two token blocks; scores row-pack two k-tiles
    (K^T tiles alternate partition halves, Q^T is duplicated); the output
    projection row-packs two query blocks against a duplicated Wo.
  - V^T -> V uses the DMA XBAR transpose, not the PE.
  - softmax denominators ride as a ones-column through the AV matmul;
    normalization happens on the host (outputs are unnormalized projected
    values plus per-token sums).
  - the output projection for a query-block pair is emitted as soon as
    its O^T halves land, so write-out overlaps attention.
  - a dummy-matmul warmup stream runs during the input DMA head to lift
    the PE HAM clock gate before real matmuls start.
"""

import sys
import types

import numpy as np
import ml_dtypes

_D = 512
_HD = 64
_H = 8
_S = 196
_NCORES = 8

_prog_cache = {}


def _ensure_ntff_hook():
    """Register the NTFF profile hook that the agent image's antenv lacks."""
    if "antenv.axon_hooks" in sys.modules:
        return
    try:
        from trn_agent_boot.trn_boot import _ntff_profile_via_ctypes

        hook = _ntff_profile_via_ctypes("/opt/axon/libaxon_pjrt.so")
        mod = types.ModuleType("antenv.axon_hooks")
        mod.get_axon_ntff_profile_hook = lambda: hook
        sys.modules["antenv.axon_hooks"] = mod
    except Exception:
        pass


def _segments(channels):
    """Return (seg_len, seg_pad, seg_off, TP) for the padded token axis."""
    seg_len = [int(c) * _S for c in np.asarray(channels).tolist() if int(c) > 0]
    seg_pad = [(l + 127) // 128 * 128 for l in seg_len]
    TP = sum(seg_pad)
    TP = (TP + 511) // 512 * 512
    seg_off = []
    o = 0
    for p in seg_pad:
        seg_off.append(o)
        o += p
    return seg_len, seg_pad, seg_off, TP


def _plan(seg_len, seg_pad, seg_off, TP):
    """Static schedule metadata shared by program build and host prep."""
    nseg = len(seg_len)
    NTB = TP // 512
    NKT = TP // 128
    nkt_s = [p // 128 for p in seg_pad]
    start_tile = [o // 128 for o in seg_off]

    g2seg = [None] * NKT
    for s in range(nseg):
        for lk in range(nkt_s[s]):
            g2seg[start_tile[s] + lk] = (s, lk)

    slot_base = []
    b = 0
    for s in range(nseg):
        slot_base.append(b)
        b += (nkt_s[s] + 1) // 2
    NSLOT = b

    w = [seg_pad[s] * seg_len[s] for s in range(nseg)]
    seg_order = sorted(range(nseg), key=lambda s: -w[s])

    def blocks_of(s):
        lo = seg_off[s] // 512
        hi = (seg_off[s] + seg_pad[s] + 511) // 512
        return list(range(lo, hi))

    qblocks = []  # (seg, qb0, qbw)
    for s in seg_order:
        for qb0 in range(0, seg_len[s], 512):
            qblocks.append((s, qb0, min(512, seg_len[s] - qb0)))
    NQB = len(qblocks)
    NPAIR = (NQB + 1) // 2

    return dict(
        nseg=nseg, NTB=NTB, NKT=NKT, nkt_s=nkt_s, start_tile=start_tile,
        g2seg=g2seg, slot_base=slot_base, NSLOT=NSLOT, seg_order=seg_order,
        blocks_of=blocks_of, qblocks=qblocks, NQB=NQB, NPAIR=NPAIR,
    )


def _build_program(seg_key, bq_nonzero):
    import concourse.bacc as bacc
    import concourse.tile as tile
    from concourse import mybir

    seg_len, seg_pad, seg_off, TP = seg_key
    seg_len, seg_pad, seg_off = list(seg_len), list(seg_pad), list(seg_off)
    P = _plan(seg_len, seg_pad, seg_off, TP)
    nseg, NTB, NKT = P["nseg"], P["NTB"], P["NKT"]
    nkt_s, start_tile, g2seg = P["nkt_s"], P["start_tile"], P["g2seg"]
    slot_base, NSLOT = P["slot_base"], P["NSLOT"]
    seg_order, blocks_of = P["seg_order"], P["blocks_of"]
    qblocks, NPAIR = P["qblocks"], P["NPAIR"]

    f32 = mybir.dt.float32
    bf16 = mybir.dt.bfloat16
    f8e4 = mybir.dt.float8e4

    nc = bacc.Bacc("TRN2", target_bir_lowering=False, debug=False,
                   num_devices=_NCORES)

    xqp_d = nc.dram_tensor("xqp", [NTB, 128, 4, 512], bf16, kind="ExternalInput")
    xkp_d = nc.dram_tensor("xkp", [NTB, 128, 4, 512], f8e4, kind="ExternalInput")
    xk_d = nc.dram_tensor("xk", [NTB, 128, 4, 512], bf16, kind="ExternalInput")
    wq_d = nc.dram_tensor("wq", [128, 4, _HD], bf16, kind="ExternalInput")
    wk_d = nc.dram_tensor("wk", [128, 4, _HD], bf16, kind="ExternalInput")
    wv_d = nc.dram_tensor("wv", [128, 4, _HD], bf16, kind="ExternalInput")
    wo2_d = nc.dram_tensor("wo2", [128, 4, 128], bf16, kind="ExternalInput")
    if bq_nonzero:
        vscale_d = nc.dram_tensor("vscale", [128, NKT], f32, kind="ExternalInput")
    outT_d = nc.dram_tensor("outT", [_D, TP], bf16, kind="ExternalOutput")
    sums_d = nc.dram_tensor("sums", [1, TP], f32, kind="ExternalOutput")

    with tile.TileContext(nc) as tc:
        with (
            tc.tile_pool(name="consts", bufs=1) as consts,
            tc.tile_pool(name="persist", bufs=1) as persist,
            tc.tile_pool(name="xin", bufs=NTB) as xin,
            tc.tile_pool(name="sbw", bufs=2) as sbw,
            tc.tile_pool(name="expp", bufs=3) as expp,
            tc.tile_pool(name="fins", bufs=4) as fins,
            tc.tile_pool(name="scA", bufs=1, space="PSUM") as scA,
            tc.tile_pool(name="scB", bufs=1, space="PSUM") as scB,
            tc.tile_pool(name="ops", bufs=1, space="PSUM") as opsp,
        ):
            wq_sb = consts.tile([128, 4, _HD], bf16)
            wk_sb = consts.tile([128, 4, _HD], bf16)
            wv_sb = consts.tile([128, 4, _HD], bf16)
            wo2_sb = consts.tile([128, 4, 128], bf16)
            nc.sync.dma_start(out=wq_sb, in_=wq_d[:, :, :])
            nc.sync.dma_start(out=wk_sb, in_=wk_d[:, :, :])
            nc.sync.dma_start(out=wv_sb, in_=wv_d[:, :, :])
            nc.sync.dma_start(out=wo2_sb, in_=wo2_d[:, :, :])
            if bq_nonzero:
                vscale_sb = consts.tile([128, NKT], f32)
                nc.sync.dma_start(out=vscale_sb, in_=vscale_d[:, :])
            wdummy = consts.tile([128, 128], bf16)
            nc.vector.memset(wdummy, 0.125)

            # persistent per-head tensors
            q2_sb = persist.tile([128, TP], bf16)          # Q^T duplicated halves
            k2_sb = persist.tile([128, NSLOT, 128], bf16)  # K^T tiles, paired halves
            v_all = persist.tile([128, NKT, _HD + 1], bf16)  # V rows + ones col
            o2_sb = persist.tile([128, NPAIR, 512], bf16)  # unnormalized O^T pairs
            sums_sb = persist.tile([1, TP], f32)

            # ones column (no data deps: disjoint from the V writes)
            for s in range(nseg):
                g0 = start_tile[s]
                nfull = seg_len[s] // 128
                rem = seg_len[s] - nfull * 128
                if nfull:
                    nc.vector.memset(v_all[:, g0:g0 + nfull, _HD:_HD + 1], 1.0)
                for lk in range(nfull, nkt_s[s]):
                    nc.vector.memset(v_all[:, g0 + lk, _HD:_HD + 1], 0.0)
                if rem:
                    nc.vector.memset(v_all[0:rem, g0 + nfull, _HD:_HD + 1], 1.0)

            # ---------------- emission helpers ----------------
            pools = {}

            def emit_qk_block(tb):
                xq_t = xin.tile([128, 4, 512], bf16, tag="xq", name="xq")
                xkp_t = xin.tile([128, 4, 512], f8e4, tag="xkp", name="xkp")
                nc.scalar.dma_start(out=xq_t, in_=xqp_d[tb])
                nc.sync.dma_start(out=xkp_t, in_=xkp_d[tb])
                ts = slice(tb * 512, (tb + 1) * 512)
                qk_ps = pools["pqk"].tile([128, 512], f32, tag="qkps", name="qkps")
                for c in range(4):
                    nc.tensor.matmul(
                        qk_ps[0:64, :], lhsT=wq_sb[:, c], rhs=xq_t[:, c],
                        start=(c == 0), stop=(c == 3),
                    )
                    nc.tensor.matmul(
                        qk_ps[64:128, :], lhsT=wk_sb[:, c], rhs=xkp_t[:, c],
                        start=(c == 0), stop=(c == 3),
                    )
                nc.vector.tensor_copy(out=q2_sb[0:64, ts], in_=qk_ps[0:64, :])
                nc.gpsimd.tensor_copy(out=q2_sb[64:128, ts], in_=q2_sb[0:64, ts])
                tiles = []
                for i in range(4):
                    g = tb * 4 + i
                    if g2seg[g] is None:
                        continue
                    s, lk = g2seg[g]
                    tiles.append((i, slot_base[s] + lk // 2, lk % 2))
                done = set()
                for idx, (i, slot, half) in enumerate(tiles):
                    if i in done:
                        continue
                    mate = next(
                        (j for (j, sl2, h2) in tiles[idx + 1:]
                         if j == i + 2 and h2 == half and sl2 == slot + 1),
                        None,
                    )
                    if mate is not None:
                        done.add(mate)
                        nc.vector.tensor_copy(
                            out=k2_sb[64 * half:64 * half + 64, slot:slot + 2, :],
                            in_=qk_ps[64:128, :].rearrange(
                                "p (b t) -> p b t", b=4, t=128)[:, i:i + 3:2, :],
                        )
                    else:
                        nc.vector.tensor_copy(
                            out=k2_sb[64 * half:64 * half + 64, slot, :],
                            in_=qk_ps[64:128, i * 128:(i + 1) * 128],
                        )

            def emit_vpair(ba, ta, bb, tb_):
                vt_ps = pools["pvt"].tile([128, 512], f32, tag="vtf", name="vtps")
                for c in range(4):
                    nc.tensor.matmul(
                        vt_ps[0:64, :], lhsT=wv_sb[:, c], rhs=ta[:, c],
                        start=(c == 0), stop=(c == 3),
                    )
                    nc.tensor.matmul(
                        vt_ps[64:128, :], lhsT=wv_sb[:, c], rhs=tb_[:, c],
                        start=(c == 0), stop=(c == 3),
                    )
                vt_sb = sbw.tile([128, 512], bf16, tag="vtsb", name="vtsb")
                nc.vector.tensor_copy(out=vt_sb, in_=vt_ps)
                stag = sbw.tile([128, 4, 128], bf16, tag="stag", name="stag")
                nc.sync.dma_start_transpose(stag, vt_sb)
                nc.gpsimd.tensor_copy(
                    out=v_all[:, 4 * ba:4 * ba + 4, 0:_HD], in_=stag[:, :, 0:64])
                nc.gpsimd.tensor_copy(
                    out=v_all[:, 4 * bb:4 * bb + 4, 0:_HD], in_=stag[:, :, 64:128])

            def emit_vlone(ba, ta):
                vt_ps = pools["pvt"].tile([128, 512], f32, tag="vtf", name="vtps")
                for c in range(4):
                    nc.tensor.matmul(
                        vt_ps[0:64, :], lhsT=wv_sb[:, c], rhs=ta[:, c],
                        start=(c == 0), stop=(c == 3),
                    )
                vt_sb1 = sbw.tile([64, 512], bf16, tag="vtsb1", name="vtsb1")
                nc.vector.tensor_copy(out=vt_sb1, in_=vt_ps[0:64, :])
                stag1 = sbw.tile([128, 4, 64], bf16, tag="stag1", name="stag1")
                nc.sync.dma_start_transpose(stag1, vt_sb1)
                nc.gpsimd.tensor_copy(
                    out=v_all[:, 4 * ba:4 * ba + 4, 0:_HD], in_=stag1[:, :, :])

            vpend = []

            def emit_xk_block(tb):
                xk_t = xin.tile([128, 4, 512], bf16, tag="xk", name="xk")
                nc.sync.dma_start(out=xk_t, in_=xk_d[tb])
                vpend.append((tb, xk_t))
                if len(vpend) == 2:
                    emit_vpair(vpend[0][0], vpend[0][1], vpend[1][0], vpend[1][1])
                    vpend.clear()

            def flush_vpend():
                # V must be emitted before attention that reads it: the shared
                # V/outproj psum slot chain would deadlock otherwise.
                if vpend:
                    emit_vlone(vpend[0][0], vpend[0][1])
                    vpend.clear()

            qb_meta = []  # (seg, qb0, qbw, goff, pslot, half)
            for i, (s, qb0, qbw) in enumerate(qblocks):
                qb_meta.append((s, qb0, qbw, seg_off[s] + qb0, i // 2, i % 2))

            exp_state = {"use_a": True}

            def emit_attention_qb(qi):
                s, qb0, qbw, goff, pslot, half = qb_meta[qi]
                nkt = nkt_s[s]
                g0 = start_tile[s]
                qcol = slice(goff, goff + qbw)
                ex = expp.tile([128, max(nkt_s), 512], bf16, tag="ex", name="ex")
                lk = 0
                while lk < nkt:
                    use_a = exp_state["use_a"]
                    cap = 3 if use_a else 2
                    G = min(cap, nkt - lk)
                    pool = scA if use_a else scB
                    sc = pool.tile(
                        [128, 3 if use_a else 2, 512], f32,
                        tag="scA" if use_a else "scB", name="sc",
                    )
                    exp_state["use_a"] = not use_a
                    for j in range(G):
                        l = lk + j
                        slot = slot_base[s] + l // 2
                        hh = l % 2
                        nc.tensor.matmul(
                            sc[:, j, 0:qbw],
                            lhsT=k2_sb[64 * hh:64 * hh + 64, slot, :],
                            rhs=q2_sb[64 * hh:64 * hh + 64, qcol],
                            start=True, stop=True,
                        )
                    nc.scalar.activation(
                        out=ex[:, lk:lk + G, 0:qbw],
                        in_=sc[:, 0:G, 0:qbw],
                        func=mybir.ActivationFunctionType.Exp,
                        scale=0.125,
                    )
                    lk += G
                # AV with ones column: O^T rows 0..63, sums in row 64
                o_ps = opsp.tile([_HD + 1, 512], f32, tag="ops", name="ops")
                for l in range(nkt):
                    nc.tensor.matmul(
                        o_ps[:, 0:qbw],
                        lhsT=v_all[:, g0 + l, :],
                        rhs=ex[:, l, 0:qbw],
                        start=(l == 0), stop=(l == nkt - 1),
                    )
                nc.vector.tensor_copy(
                    out=o2_sb[64 * half:64 * half + 64, pslot, 0:qbw],
                    in_=o_ps[0:_HD, 0:qbw],
                )
                nc.vector.tensor_copy(
                    out=sums_sb[0:1, qcol], in_=o_ps[_HD:_HD + 1, 0:qbw])

            # ---------------- emission stream ----------------
            # scope B: projection psum pools; close mid-kernel so the
            # outproj pool (scope C) reuses their banks without touching
            # the long-lived score/AV pools.
            with (
                tc.tile_pool(name="pqk", bufs=1, space="PSUM") as pqk_,
                tc.tile_pool(name="pvt", bufs=1, space="PSUM") as pvt_,
            ):
                pools["pqk"] = pqk_
                pools["pvt"] = pvt_
                # HAM warmup through the DMA head (rotates the V psum slot)
                warm_ps = pvt_.tile([128, 512], f32, tag="vtf", name="warm")
                for _ in range(30):
                    nc.tensor.matmul(warm_ps[:, 0:128], lhsT=wdummy,
                                     rhs=wdummy, start=True, stop=True)

                qk_done = set()
                xk_done = set()
                qi = 0
                for s in seg_order:
                    for tb in blocks_of(s):
                        if tb not in qk_done:
                            qk_done.add(tb)
                            emit_qk_block(tb)
                    for tb in blocks_of(s):
                        if tb not in xk_done:
                            xk_done.add(tb)
                            emit_xk_block(tb)
                    flush_vpend()
                    if bq_nonzero:
                        for lk in range(nkt_s[s]):
                            g = start_tile[s] + lk
                            nc.vector.tensor_scalar_mul(
                                v_all[:, g, :], v_all[:, g, :],
                                vscale_sb[:, g:g + 1])
                    while qi < len(qb_meta) and qb_meta[qi][0] == s:
                        emit_attention_qb(qi)
                        qi += 1
                flush_vpend()
                while qi < len(qb_meta):
                    emit_attention_qb(qi)
                    qi += 1

            # scope C: output projection (banks reused from scope B)
            with tc.tile_pool(name="finp", bufs=2, space="PSUM") as finp:
                for pslot in range(NPAIR):
                    members = [m for m in qb_meta if m[4] == pslot]
                    f4 = {}
                    for (s, qb0, qbw, goff, _, half) in members:
                        f4[half] = fins.tile([128, 4, 512], bf16, tag="fsb",
                                             name="fsb")
                    for c in range(4):
                        for (s, qb0, qbw, goff, _, half) in members:
                            f_ps = finp.tile([128, 512], f32, tag="fin",
                                             name="fin")
                            nc.tensor.matmul(
                                f_ps[:, 0:qbw],
                                lhsT=wo2_sb[64 * half:64 * half + 64, c, :],
                                rhs=o2_sb[64 * half:64 * half + 64, pslot,
                                          0:qbw],
                                start=True, stop=True,
                            )
                            nc.vector.tensor_copy(
                                out=f4[half][:, c, 0:qbw], in_=f_ps[:, 0:qbw])
                    for (s, qb0, qbw, goff, _, half) in members:
                        nc.gpsimd.dma_start(
                            out=outT_d.rearrange("(c p) t -> p c t", p=128)[
                                :, :, goff:goff + qbw],
                            in_=f4[half][:, :, 0:qbw],
                        )
            nc.gpsimd.dma_start(out=sums_d[:, :], in_=sums_sb[:, :])

    nc.compile()
    return nc


def _prep_blocked(x, seg_len, seg_pad, seg_off, TP, dtype):
    """[T, D] f32 -> [NTB, 128, 4, 512] with per-segment padding.

    Element (tb, p, c, t) = x[tb*512 + t (padded axis), c*128 + p].
    """
    xp = np.zeros((TP, _D), dtype=np.float32)
    o = 0
    for l, off in zip(seg_len, seg_off):
        xp[off:off + l] = x[o:o + l]
        o += l
    xt = xp.T.reshape(4, 128, TP // 512, 512)  # [c, p, tb, t]
    return np.ascontiguousarray(xt.transpose(2, 1, 0, 3)).astype(dtype)


def kernel(x_query, x_keyval, pos, channels, Wq, bq, Wk, bk, Wv, bv, Wo, bo,
           _trace=False, _trace_cores=None):
    _ensure_ntff_hook()
    import concourse.bass_utils as bu

    bu.upload_artifacts = lambda tmpdir: tmpdir  # no S3 egress from here

    x_query = np.asarray(x_query, dtype=np.float32)
    x_keyval = np.asarray(x_keyval, dtype=np.float32)
    pos = np.asarray(pos, dtype=np.float32)
    channels = np.asarray(channels)
    Wq, bq = np.asarray(Wq, np.float32), np.asarray(bq, np.float32)
    Wk, bk = np.asarray(Wk, np.float32), np.asarray(bk, np.float32)
    Wv, bv = np.asarray(Wv, np.float32), np.asarray(bv, np.float32)
    Wo, bo = np.asarray(Wo, np.float32), np.asarray(bo, np.float32)

    C, S, D = x_query.shape
    seg_len, seg_pad, seg_off, TP = _segments(channels)
    assert sum(seg_len) == C * S, "channels inconsistent with batch dim"
    P = _plan(seg_len, seg_pad, seg_off, TP)

    bq_nonzero = bool(np.any(bq))
    seg_key = (tuple(seg_len), tuple(seg_pad), tuple(seg_off), TP)
    cache_key = (seg_key, bq_nonzero)
    if cache_key not in _prog_cache:
        _prog_cache[cache_key] = _build_program(seg_key, bq_nonzero)
    nc = _prog_cache[cache_key]

    bf = ml_dtypes.bfloat16
    f8 = ml_dtypes.float8_e4m3fn
    xq_flat = x_query.reshape(-1, D)
    xk_flat = x_keyval.reshape(-1, D)
    p_flat = pos.reshape(-1, D)
    xqp_b = _prep_blocked(xq_flat + p_flat, seg_len, seg_pad, seg_off, TP, bf)
    xkp_b = _prep_blocked(xk_flat + p_flat, seg_len, seg_pad, seg_off, TP, f8)
    xk_b = _prep_blocked(xk_flat, seg_len, seg_pad, seg_off, TP, bf)

    def wchunk(w):  # [512, 64] -> [128, 4, 64]
        return np.ascontiguousarray(
            w.reshape(4, 128, _HD).transpose(1, 0, 2)).astype(bf)

    in_maps = []
    for h in range(_NCORES):
        sl = slice(h * _HD, (h + 1) * _HD)
        wo_h = Wo[sl, :].reshape(_HD, 4, 128)  # [64, c, j]
        wo2 = np.empty((128, 4, 128), dtype=np.float32)
        wo2[0:64] = wo_h
        wo2[64:128] = wo_h
        m = {
            "xqp": xqp_b,
            "xkp": xkp_b,
            "xk": xk_b,
            "wq": wchunk(Wq[:, sl]),
            "wk": wchunk(Wk[:, sl]),
            "wv": wchunk(Wv[:, sl]),
            "wo2": np.ascontiguousarray(wo2).astype(bf),
        }
        if bq_nonzero:
            # exact bq folding: scale V rows by exp((xkp @ Wk_h @ bq_h)/8)
            # (uses the same fp8-quantized xkp the device sees)
            xkp_q = (xk_flat + p_flat).astype(f8).astype(np.float32)
            bqk = xkp_q @ (
                Wk[:, sl].astype(bf).astype(np.float32) @ bq[sl])  # [T]
            f = np.zeros(TP, dtype=np.float64)
            o = 0
            for l, off in zip(seg_len, seg_off):
                f[off:off + l] = np.exp(bqk[o:o + l] / 8.0)
                o += l
            m["vscale"] = np.ascontiguousarray(
                f.reshape(P["NKT"], 128).T).astype(np.float32)
        in_maps.append(m)

    from concourse.bass_utils import run_bass_kernel_spmd

    kwargs = {}
    if _trace:
        kwargs["trace"] = True
        if _trace_cores is not None:
            kwargs["trace_cores"] = _trace_cores
    res = run_bass_kernel_spmd(nc, in_maps, list(range(_NCORES)), **kwargs)

    # host gather: per-head softmax normalization + sum + bias folds
    acc = np.zeros((_D, TP), dtype=np.float64)
    with np.errstate(divide="ignore", invalid="ignore"):
        for h in range(_NCORES):
            outT = np.asarray(res.results[h]["outT"], dtype=np.float64)
            sums = np.asarray(res.results[h]["sums"], dtype=np.float64)[0]
            sums = np.where(sums == 0.0, 1.0, sums)
            acc += outT / sums[None, :]

    const = bv @ Wo + bo  # bias fold (exact; zero in the spec's fills)

    out = np.empty((C * S, D), dtype=np.float32)
    o = 0
    for l, off in zip(seg_len, seg_off):
        out[o:o + l] = acc[:, off:off + l].T
        o += l
    out = (out + const[None, :]).astype(np.float32).reshape(C, S, D)

    if _trace:
        kernel._last_exec_time_ns = res.exec_time_ns
        kernel._last_trace = (
            res.instructions_and_trace[1] if res.instructions_and_trace else None
        )
    return out
